# revision 6
# baseline (speedup 1.0000x reference)
"""CTC loss kernel for Trainium2 (8 NeuronCores, SPMD data-parallel over batch).

Strategy:
  - Shard batch B=32 across 8 cores (4 sequences each).
  - Phase 1 (memory-bound): stream logits tiles [128 t x 6000 c]; ACT computes
    exp + row-sum in one pass; GPSIMD ap_gather pulls the 65 extended-label
    columns; DVE normalizes to probabilities p and masked q = p*allow.
    p/q staged to DRAM and reloaded in a [4b x (t,s)] layout.
  - Phase 2: CTC alpha recurrence in probability domain with periodic
    rescaling (scale factors tracked, logs summed at the end).  Split into a
    forward chain (t=0..255) and a backward chain (t=511..256) which meet in
    the middle: P(l|x) = sum_s alpha_255(s) * beta_255(s).
  - Host: mean over batch / L, zero_infinity semantics.
"""

import os
import sys
from contextlib import ExitStack

import numpy as np

for _p in ("/opt/trn_rl_repo", "/root/.axon_site/_ro/trn_rl_repo"):
    if os.path.isdir(_p) and _p not in sys.path:
        sys.path.insert(0, _p)

T, B, C, L = 512, 32, 6000, 32
NCORES = 8
BS = B // NCORES            # 4 sequences per core
S = 2 * L + 1               # 65 extended-label positions
SPAD = 80                   # gather count padded to a multiple of 16
NIDX_COLS = SPAD // 16      # 5
TCH = 128                   # phase-1 t-chunk (tile partition rows)
DCH = 64                    # DP chunk (t steps per staged PQ chunk)
WCOL = 2 * S                # 130 staged columns per t (p then q)
RESC = 4                    # rescale period (steps)
MBIAS = 1e30                # rescale bias: keeps fp32 window well above flush
NSC = 1 + len(range(RESC, T - RESC + 1, RESC))  # applied scale count (init + rescales)

_CACHE = {}


def _build_program():
    import concourse.tile as tile
    from concourse import bacc, mybir

    f32 = mybir.dt.float32
    i16 = mybir.dt.int16
    AF = mybir.ActivationFunctionType
    OP = mybir.AluOpType
    AX = mybir.AxisListType

    nc = bacc.Bacc(
        "TRN2",
        target_bir_lowering=False,
        debug=False,
        enable_asserts=False,
        num_devices=NCORES,
    )

    x_in = nc.declare_dram_parameter("x", [T, BS, C], f32, isOutput=False)
    gidx_in = nc.declare_dram_parameter(
        "gidx", [BS, 128, NIDX_COLS], i16, isOutput=False
    )
    allow_in = nc.declare_dram_parameter("allow", [BS, 128, S], f32, isOutput=False)
    loss_out = nc.declare_dram_parameter("loss", [BS, 1], f32, isOutput=True)

    with ExitStack() as ctx:
        tc = ctx.enter_context(tile.TileContext(nc))
        sb = ctx.enter_context(tc.tile_pool(name="sb", bufs=1))
        xp = ctx.enter_context(tc.tile_pool(name="xp", bufs=2))
        gp = ctx.enter_context(tc.tile_pool(name="gp", bufs=2))
        pqp = ctx.enter_context(tc.tile_pool(name="pqp", bufs=2))
        dpq = ctx.enter_context(tc.tile_pool(name="dpq", bufs=2))
        drp = ctx.enter_context(tc.tile_pool(name="drp", bufs=1, space="DRAM"))

        pq_dram = drp.tile([BS, T * WCOL], f32, tag="pqdram")
        pqv = pq_dram[:].rearrange("b (t w) -> b t w", w=WCOL)

        gidx_t = [
            sb.tile([128, NIDX_COLS], i16, tag=f"gidx{b}", name=f"gidx{b}")
            for b in range(BS)
        ]
        allow_t = [
            sb.tile([128, S], f32, tag=f"allow{b}", name=f"allow{b}")
            for b in range(BS)
        ]
        for b in range(BS):
            nc.sync.dma_start(gidx_t[b][:], gidx_in[b, :, :])
            nc.sync.dma_start(allow_t[b][:], allow_in[b, :, :])

        def phase1_chunk(tcj):
            t0 = tcj * TCH
            for b in range(BS):
                xt = xp.tile([128, C], f32, tag="xt")
                nc.sync.dma_start(xt[:], x_in[t0 : t0 + TCH, b, :])
                z = gp.tile([128, 1], f32, tag="z")
                nc.scalar.activation(xt[:], xt[:], AF.Exp, accum_out=z[:])
                g = gp.tile([128, SPAD], f32, tag="g")
                nc.gpsimd.ap_gather(
                    g[:],
                    xt[:].rearrange("p (n d) -> p n d", d=1),
                    gidx_t[b][:],
                    channels=128,
                    num_elems=C,
                    d=1,
                    num_idxs=SPAD,
                )
                rz = gp.tile([128, 1], f32, tag="rz")
                nc.vector.reciprocal(rz[:], z[:])
                pt = pqp.tile([128, S], f32, tag="pt")
                nc.vector.tensor_scalar_mul(pt[:], g[:, 0:S], rz[:])
                qt = pqp.tile([128, S], f32, tag="qt")
                nc.vector.tensor_mul(qt[:], pt[:], allow_t[b][:])
                nc.scalar.dma_start(pqv[b, t0 : t0 + TCH, 0:S], pt[:])
                nc.scalar.dma_start(pqv[b, t0 : t0 + TCH, S:WCOL], qt[:])

        pq_tiles = {}

        def load_dp_chunk(c, side):
            tl = dpq.tile([BS, DCH * WCOL], f32, tag=f"dp_{side}")
            nc.scalar.dma_start(
                tl[:], pq_dram[:, c * DCH * WCOL : (c + 1) * DCH * WCOL]
            )
            pq_tiles[c] = tl

        # Natural order: DP consumes chunks 0..7 as phase 1 produces them.
        for tcj in range(T // TCH):
            phase1_chunk(tcj)
            load_dp_chunk(2 * tcj, "f")
            load_dp_chunk(2 * tcj + 1, "f")

        # DP state tiles.  Layout (134 cols): [0..64] u-scratch, [65,66]
        # low guards (alpha(-2), alpha(-1), stay zero), [67..131] s=0..64,
        # [132,133] unused.  The key trick: cols [0..129] then read as
        # [u(65) | alpha_shift2(65)] contiguously, so one wide multiply
        # against the staged [p|q] row computes both products of the step.
        SW = 134
        A0 = sb.tile([BS, SW], f32, tag="A0")
        A1 = sb.tile([BS, SW], f32, tag="A1")
        XW = sb.tile([BS, 2 * S], f32, tag="XW")
        Sf = sb.tile([BS, 130], f32, tag="Sf")
        rF = sb.tile([BS, 1], f32, tag="rF")
        rT = sb.tile([BS, 1], f32, tag="rT")

        for tl_ in (A0, A1):
            nc.vector.memset(tl_[:], 0.0)
        nc.vector.memset(Sf[:], 1.0)

        def pslice(t):
            c = t // DCH
            dt = t - c * DCH
            tl = pq_tiles[c]
            return tl[:, dt * WCOL : dt * WCOL + WCOL]

        # forward init at MBIAS scale: alpha_0(s) = MBIAS * p_0(s), s in {0,1}
        nc.vector.tensor_scalar_mul(A0[:, 67:69], pslice(0)[:, 0:2], MBIAS)

        # ---- forward chain: steps k=1..511 consume p_k ----
        cur, nxt = A0, A1
        for k in range(1, T):
            pq_s = pslice(k)
            acc = (k % RESC == 0) and k + 1 < T
            apply = (k % RESC == 1) and k > 2
            sidx = k // RESC
            # u = a1 + a2 into scratch cols [0:65] of the current tile
            nc.vector.tensor_add(cur[:, 0:S], cur[:, 67 : 67 + S], cur[:, 66 : 66 + S])
            # XW = [u | a3] * [p | q]  (one wide multiply)
            if apply:
                nc.vector.scalar_tensor_tensor(
                    XW[:], cur[:, 0 : 2 * S], rF[:], pq_s, OP.mult, OP.mult
                )
            elif acc:
                nc.vector.scalar_tensor_tensor(
                    XW[:],
                    cur[:, 0 : 2 * S],
                    1.0,
                    pq_s,
                    OP.mult,
                    OP.mult,
                    accum_out=Sf[:, sidx : sidx + 1],
                )
            else:
                nc.vector.tensor_mul(XW[:], cur[:, 0 : 2 * S], pq_s)
            # alpha' = x + w
            nc.vector.tensor_add(nxt[:, 67 : 67 + S], XW[:, 0:S], XW[:, S : 2 * S])
            if acc:
                nc.vector.reciprocal(rT[:], Sf[:, sidx : sidx + 1])
                nc.vector.tensor_scalar_mul(rF[:], rT[:], MBIAS)
            cur, nxt = nxt, cur
        a_fin = cur

        # ---- finalize: P ~ alpha_511(64) + alpha_511(63), plus log scales
        L0 = sb.tile([BS, 1], f32, tag="L0")
        nc.vector.tensor_add(L0[:], a_fin[:, 130:131], a_fin[:, 131:132])
        lgL = sb.tile([BS, 1], f32, tag="lgL")
        nc.scalar.activation(lgL[:], L0[:], AF.Ln)
        lgF = sb.tile([BS, 130], f32, tag="lgF")
        nc.scalar.activation(lgF[:], Sf[:], AF.Ln)
        r1 = sb.tile([BS, 1], f32, tag="r1")
        nc.vector.tensor_reduce(r1[:], lgF[:], axis=AX.X, op=OP.add)
        tot = sb.tile([BS, 1], f32, tag="tot")
        nc.vector.tensor_add(tot[:], lgL[:], r1[:])
        nloss = sb.tile([BS, 1], f32, tag="nloss")
        nc.vector.tensor_scalar_mul(nloss[:], tot[:], -1.0)
        nc.sync.dma_start(loss_out[:, :], nloss[:])

    return nc


def get_program():
    if "nc" not in _CACHE:
        nc = _build_program()
        nc.compile()
        _CACHE["nc"] = nc
    return _CACHE["nc"]


def make_in_maps(input_np, label_np):
    inp = np.asarray(input_np, dtype=np.float32)
    lab = np.asarray(label_np).astype(np.int64)
    assert inp.shape == (T, B, C) and lab.shape == (B, L)
    assert (lab >= 0).all(), "padding (-1) labels not supported by this kernel"
    in_maps = []
    for core in range(NCORES):
        b0 = core * BS
        xs = np.ascontiguousarray(inp[:, b0 : b0 + BS, :])
        labs = lab[b0 : b0 + BS]
        ext = np.zeros((BS, S), np.int64)
        ext[:, 1::2] = labs + 1
        allow = np.zeros((BS, S), np.float32)
        allow[:, 3::2] = (labs[:, 1:] != labs[:, :-1]).astype(np.float32)
        gidx = np.zeros((BS, 128, NIDX_COLS), np.int16)
        for j in range(SPAD):
            v = ext[:, j].astype(np.int16) if j < S else np.zeros(BS, np.int16)
            for kq in range(8):
                gidx[:, 16 * kq + (j % 16), j // 16] = v
        allow_rep = np.ascontiguousarray(
            np.broadcast_to(allow[:, None, :], (BS, 128, S))
        )
        in_maps.append({"x": xs, "gidx": gidx, "allow": allow_rep})
    return in_maps


def finalize(results):
    losses = np.concatenate(
        [np.asarray(r["loss"], np.float32).reshape(-1) for r in results]
    )
    losses = losses + np.float32(NSC * np.log(MBIAS))
    losses = np.where(np.isfinite(losses) & (losses < 1e29), losses, np.float32(0.0))
    return np.float32(losses.mean() / np.float32(L))


def kernel(input, label):
    from concourse.bass_utils import run_bass_kernel_spmd

    nc = get_program()
    in_maps = make_in_maps(input, label)
    res = run_bass_kernel_spmd(nc, in_maps, list(range(NCORES)))
    return finalize(res.results)


if __name__ == "__main__":
    rng = np.random.default_rng(0)
    x = rng.standard_normal((T, B, C), dtype=np.float32)
    lab = rng.integers(0, 5999, size=(B, L), dtype=np.int64)
    print(kernel(x, lab))


# revision 8
# speedup vs baseline: 1.2681x; 1.2681x over previous
"""CTC loss kernel for Trainium2 (8 NeuronCores, SPMD data-parallel over batch).

Strategy:
  - Shard batch B=32 across 8 cores (4 sequences each).
  - Phase 1 (memory-bound): stream logits tiles [128 t x 6000 c]; ACT computes
    exp + row-sum in one pass; GPSIMD ap_gather pulls the 65 extended-label
    columns; DVE normalizes to probabilities p and masked q = p*allow*lam^2.
    p/q staged to DRAM and reloaded in a [4b x (t,s)] layout.
  - Phase 2: CTC alpha recurrence in probability domain, tilted by
    exp(-g*s) per position so the fp32 dynamic-range window covers the whole
    vector, with periodic rescaling (scale factors tracked, logs summed at
    the end).  Two independent chains — forward (t=0..255) and backward
    (t=511..256) — interleave on the vector engine to hide the per-op
    dependent-issue latency; equal tilts cancel in the meeting dot product:
    P(l|x) = exp(64 g) * sum_s alpha~_255(s) * beta~_255(s).
  - Host: mean over batch / L, zero_infinity semantics.
"""

import os
import sys
from contextlib import ExitStack

import numpy as np

for _p in ("/opt/trn_rl_repo", "/root/.axon_site/_ro/trn_rl_repo"):
    if os.path.isdir(_p) and _p not in sys.path:
        sys.path.insert(0, _p)

T, B, C, L = 512, 32, 6000, 32
NCORES = 8
BS = B // NCORES            # 4 sequences per core
S = 2 * L + 1               # 65 extended-label positions
SPAD = 80                   # gather count padded to a multiple of 16
NIDX_COLS = SPAD // 16      # 5
TCH = 128                   # phase-1 t-chunk (tile partition rows)
DCH = 64                    # DP chunk (t steps per staged PQ chunk)
WCOL = 2 * S                # 130 staged columns per t
RESC = 4                    # rescale period (steps)
MBIAS = 1e17                # rescale bias: keeps fp32 window well above flush
GTILT = 2.9                 # per-position tilt (nats per s)
LAM = float(np.exp(-GTILT))
NSC = 128                   # applied scale count (2 chains x (1 init + 63))
THALF = T // 2              # 256; fwd consumes p_1..p_255, bwd p_511..p_256

_CACHE = {}


def _build_program():
    import concourse.tile as tile
    from concourse import bacc, mybir

    f32 = mybir.dt.float32
    i16 = mybir.dt.int16
    AF = mybir.ActivationFunctionType
    OP = mybir.AluOpType
    AX = mybir.AxisListType

    nc = bacc.Bacc(
        "TRN2",
        target_bir_lowering=False,
        debug=False,
        enable_asserts=False,
        num_devices=NCORES,
    )

    x_in = nc.declare_dram_parameter("x", [T, BS, C], f32, isOutput=False)
    gidx_in = nc.declare_dram_parameter(
        "gidx", [BS, 128, NIDX_COLS], i16, isOutput=False
    )
    allow_in = nc.declare_dram_parameter("allow", [BS, 128, S], f32, isOutput=False)
    loss_out = nc.declare_dram_parameter("loss", [BS, 1], f32, isOutput=True)

    with ExitStack() as ctx:
        tc = ctx.enter_context(tile.TileContext(nc))
        sb = ctx.enter_context(tc.tile_pool(name="sb", bufs=1))
        xp = ctx.enter_context(tc.tile_pool(name="xp", bufs=2))
        gp = ctx.enter_context(tc.tile_pool(name="gp", bufs=2))
        pqp = ctx.enter_context(tc.tile_pool(name="pqp", bufs=2))
        dpq = ctx.enter_context(tc.tile_pool(name="dpq", bufs=2))
        drp = ctx.enter_context(tc.tile_pool(name="drp", bufs=1, space="DRAM"))

        pq_dram = drp.tile([BS, T * WCOL], f32, tag="pqdram")
        pqv = pq_dram[:].rearrange("b (t w) -> b t w", w=WCOL)

        gidx_t = [
            sb.tile([128, NIDX_COLS], i16, tag=f"gidx{b}", name=f"gidx{b}")
            for b in range(BS)
        ]
        allow_t = [
            sb.tile([128, S], f32, tag=f"allow{b}", name=f"allow{b}")
            for b in range(BS)
        ]
        for b in range(BS):
            nc.sync.dma_start(gidx_t[b][:], gidx_in[b, :, :])
            nc.sync.dma_start(allow_t[b][:], allow_in[b, :, :])

        def phase1_chunk(tcj):
            # fwd-side chunks (t<256) stage [p | q~]; bwd-side stage [q~ | p]
            fwd_side = tcj * TCH < THALF
            t0 = tcj * TCH
            for b in range(BS):
                xt = xp.tile([128, C], f32, tag="xt", name="xt")
                nc.sync.dma_start(xt[:], x_in[t0 : t0 + TCH, b, :])
                z = gp.tile([128, 1], f32, tag="z", name="z")
                nc.scalar.activation(xt[:], xt[:], AF.Exp, accum_out=z[:])
                g = gp.tile([128, SPAD], f32, tag="g", name="g")
                nc.gpsimd.ap_gather(
                    g[:],
                    xt[:].rearrange("p (n d) -> p n d", d=1),
                    gidx_t[b][:],
                    channels=128,
                    num_elems=C,
                    d=1,
                    num_idxs=SPAD,
                )
                rz = gp.tile([128, 1], f32, tag="rz", name="rz")
                nc.vector.reciprocal(rz[:], z[:])
                pt = pqp.tile([128, S], f32, tag="pt", name="pt")
                nc.vector.tensor_scalar_mul(pt[:], g[:, 0:S], rz[:])
                qt = pqp.tile([128, S], f32, tag="qt", name="qt")
                nc.vector.tensor_mul(qt[:], pt[:], allow_t[b][:])
                if fwd_side:
                    nc.scalar.dma_start(pqv[b, t0 : t0 + TCH, 0:S], pt[:])
                    nc.scalar.dma_start(pqv[b, t0 : t0 + TCH, S:WCOL], qt[:])
                else:
                    nc.scalar.dma_start(pqv[b, t0 : t0 + TCH, 0:S], qt[:])
                    nc.scalar.dma_start(pqv[b, t0 : t0 + TCH, S:WCOL], pt[:])

        pq_tiles = {}

        def load_dp_chunk(c, side):
            tl = dpq.tile(
                [BS, DCH * WCOL], f32, tag=f"dp_{side}", name=f"dpc{c}"
            )
            nc.scalar.dma_start(
                tl[:], pq_dram[:, c * DCH * WCOL : (c + 1) * DCH * WCOL]
            )
            pq_tiles[c] = tl

        # Emission order: each chain's chunks become available as its side of
        # phase 1 completes.  fwd eats chunks 0..3, bwd eats 7..4.
        phase1_chunk(0)
        load_dp_chunk(0, "f")
        load_dp_chunk(1, "f")
        phase1_chunk(3)
        load_dp_chunk(7, "b")
        load_dp_chunk(6, "b")
        phase1_chunk(1)
        load_dp_chunk(2, "f")
        load_dp_chunk(3, "f")
        phase1_chunk(2)
        load_dp_chunk(5, "b")
        load_dp_chunk(4, "b")

        # ---- DP state tiles ----
        # fwd layout (134 cols): [0..64] u, [65,66] guards, [67..131] s=0..64
        # bwd layout (134 cols): [2..66] s=0..64, [67,68] guards, [69..133] u
        SW = 134
        A0 = sb.tile([BS, SW], f32, tag="A0")
        A1 = sb.tile([BS, SW], f32, tag="A1")
        B0 = sb.tile([BS, SW], f32, tag="B0")
        B1 = sb.tile([BS, SW], f32, tag="B1")
        XWf = sb.tile([BS, 2 * S], f32, tag="XWf")
        XWb = sb.tile([BS, 2 * S], f32, tag="XWb")
        Sf = sb.tile([BS, 130], f32, tag="Sf")
        Sb = sb.tile([BS, 130], f32, tag="Sb")
        rF = sb.tile([BS, 1], f32, tag="rF")
        rB = sb.tile([BS, 1], f32, tag="rB")
        rT = sb.tile([BS, 1], f32, tag="rT")
        rU = sb.tile([BS, 1], f32, tag="rU")

        for tl_ in (A0, A1, B0, B1):
            nc.vector.memset(tl_[:], 0.0)
        nc.vector.memset(Sf[:], 1.0)
        nc.vector.memset(Sb[:], 1.0)

        def pslice(t):
            c = t // DCH
            dt = t - c * DCH
            tl = pq_tiles[c]
            return tl[:, dt * WCOL : dt * WCOL + WCOL]

        # fwd init: a~_0 = MBIAS * [p_0(0), lam*p_0(1)] at cols 67,68
        nc.vector.tensor_scalar_mul(A0[:, 67:69], pslice(0)[:, 0:2], MBIAS)
        nc.vector.tensor_scalar_mul(A0[:, 68:69], A0[:, 68:69], LAM)
        # bwd init: b~_511 = MBIAS * [lam at s=63 (col 65), 1 at s=64 (col 66)]
        nc.vector.memset(B0[:, 65:66], MBIAS * LAM)
        nc.vector.memset(B0[:, 66:67], MBIAS)

        fcur, fnxt = A0, A1
        bcur, bnxt = B0, B1

        def fwd_step(k):
            nonlocal fcur, fnxt
            cur, nxt = fcur, fnxt
            pq_s = pslice(k)
            acc = (k % RESC == 0) and k + 1 < THALF
            apply = (k % RESC == 1) and k > 2
            sidx = k // RESC
            ops = []
            ops.append(
                lambda: nc.vector.scalar_tensor_tensor(
                    cur[:, 0:S], cur[:, 66 : 66 + S], LAM,
                    cur[:, 67 : 67 + S], OP.mult, OP.add,
                )
            )
            if apply:
                ops.append(
                    lambda: nc.vector.scalar_tensor_tensor(
                        XWf[:], cur[:, 0 : 2 * S], rF[:], pq_s, OP.mult, OP.mult
                    )
                )
            elif acc:
                ops.append(
                    lambda: nc.vector.scalar_tensor_tensor(
                        XWf[:], cur[:, 0 : 2 * S], 1.0, pq_s, OP.mult, OP.mult,
                        accum_out=Sf[:, sidx : sidx + 1],
                    )
                )
            else:
                ops.append(
                    lambda: nc.vector.tensor_mul(XWf[:], cur[:, 0 : 2 * S], pq_s)
                )
            ops.append(
                lambda: nc.vector.tensor_add(
                    nxt[:, 67 : 67 + S], XWf[:, 0:S], XWf[:, S : 2 * S]
                )
            )
            post = []
            if acc:
                post.append(
                    lambda: nc.vector.reciprocal(rT[:], Sf[:, sidx : sidx + 1])
                )
                post.append(lambda: nc.vector.tensor_scalar_mul(rF[:], rT[:], MBIAS))
            fcur, fnxt = fnxt, fcur
            return ops, post

        def bwd_step(m):
            nonlocal bcur, bnxt
            cur, nxt = bcur, bnxt
            t = T - m
            pq_s = pslice(t)
            acc = (m % RESC == 0) and m + 1 < THALF + 1
            apply = (m % RESC == 1) and m > 2
            sidx = m // RESC
            ops = []
            ops.append(
                lambda: nc.vector.scalar_tensor_tensor(
                    cur[:, 69 : 69 + S], cur[:, 3 : 3 + S], LAM,
                    cur[:, 2 : 2 + S], OP.mult, OP.add,
                )
            )
            if apply:
                ops.append(
                    lambda: nc.vector.scalar_tensor_tensor(
                        XWb[:], cur[:, 4 : 4 + 2 * S], rB[:], pq_s, OP.mult, OP.mult
                    )
                )
            elif acc:
                ops.append(
                    lambda: nc.vector.scalar_tensor_tensor(
                        XWb[:], cur[:, 4 : 4 + 2 * S], 1.0, pq_s, OP.mult, OP.mult,
                        accum_out=Sb[:, sidx : sidx + 1],
                    )
                )
            else:
                ops.append(
                    lambda: nc.vector.tensor_mul(
                        XWb[:], cur[:, 4 : 4 + 2 * S], pq_s
                    )
                )
            ops.append(
                lambda: nc.vector.tensor_add(
                    nxt[:, 2 : 2 + S], XWb[:, 0:S], XWb[:, S : 2 * S]
                )
            )
            post = []
            if acc:
                post.append(
                    lambda: nc.vector.reciprocal(rU[:], Sb[:, sidx : sidx + 1])
                )
                post.append(lambda: nc.vector.tensor_scalar_mul(rB[:], rU[:], MBIAS))
            bcur, bnxt = bnxt, bcur
            return ops, post

        # zippered emission: alternate fwd/bwd ops so consecutive DVE
        # instructions are independent
        for i in range(1, THALF + 1):
            fops, fpost = fwd_step(i) if i <= THALF - 1 else ([], [])
            bops, bpost = bwd_step(i)
            for j in range(max(len(fops), len(bops))):
                if j < len(fops):
                    fops[j]()
                if j < len(bops):
                    bops[j]()
            for fn in fpost:
                fn()
            for fn in bpost:
                fn()

        a_fin, b_fin = fcur, bcur

        # ---- finalize: D = sum_s a~(s) b~(s); loss parts assembled ----
        Dt = sb.tile([BS, S], f32, tag="Dt")
        L0 = sb.tile([BS, 1], f32, tag="L0")
        nc.vector.scalar_tensor_tensor(
            Dt[:], a_fin[:, 67 : 67 + S], 1.0 / MBIAS, b_fin[:, 2 : 2 + S],
            OP.mult, OP.mult, accum_out=L0[:],
        )
        lgL = sb.tile([BS, 1], f32, tag="lgL")
        nc.scalar.activation(lgL[:], L0[:], AF.Ln)
        lgF = sb.tile([BS, 130], f32, tag="lgF")
        nc.scalar.activation(lgF[:], Sf[:], AF.Ln)
        lgB = sb.tile([BS, 130], f32, tag="lgB")
        nc.scalar.activation(lgB[:], Sb[:], AF.Ln)
        r1 = sb.tile([BS, 1], f32, tag="r1")
        nc.vector.tensor_reduce(r1[:], lgF[:], axis=AX.X, op=OP.add)
        r2 = sb.tile([BS, 1], f32, tag="r2")
        nc.vector.tensor_reduce(r2[:], lgB[:], axis=AX.X, op=OP.add)
        tot = sb.tile([BS, 1], f32, tag="tot")
        nc.vector.tensor_add(tot[:], lgL[:], r1[:])
        nc.vector.tensor_add(tot[:], tot[:], r2[:])
        nloss = sb.tile([BS, 1], f32, tag="nloss")
        nc.vector.tensor_scalar_mul(nloss[:], tot[:], -1.0)
        nc.sync.dma_start(loss_out[:, :], nloss[:])

    return nc


def get_program():
    if "nc" not in _CACHE:
        nc = _build_program()
        nc.compile()
        _CACHE["nc"] = nc
    return _CACHE["nc"]


def make_in_maps(input_np, label_np):
    inp = np.asarray(input_np, dtype=np.float32)
    lab = np.asarray(label_np).astype(np.int64)
    assert inp.shape == (T, B, C) and lab.shape == (B, L)
    assert (lab >= 0).all(), "padding (-1) labels not supported by this kernel"
    lam2 = np.float32(LAM * LAM)
    in_maps = []
    for core in range(NCORES):
        b0 = core * BS
        xs = np.ascontiguousarray(inp[:, b0 : b0 + BS, :])
        labs = lab[b0 : b0 + BS]
        ext = np.zeros((BS, S), np.int64)
        ext[:, 1::2] = labs + 1
        allow = np.zeros((BS, S), np.float32)
        allow[:, 3::2] = (labs[:, 1:] != labs[:, :-1]).astype(np.float32) * lam2
        gidx = np.zeros((BS, 128, NIDX_COLS), np.int16)
        for j in range(SPAD):
            v = ext[:, j].astype(np.int16) if j < S else np.zeros(BS, np.int16)
            for kq in range(8):
                gidx[:, 16 * kq + (j % 16), j // 16] = v
        allow_rep = np.ascontiguousarray(
            np.broadcast_to(allow[:, None, :], (BS, 128, S))
        )
        in_maps.append({"x": xs, "gidx": gidx, "allow": allow_rep})
    return in_maps


def finalize(results):
    losses = np.concatenate(
        [np.asarray(r["loss"], np.float32).reshape(-1) for r in results]
    )
    losses = losses + np.float32((NSC - 1) * np.log(MBIAS) - 64.0 * GTILT)
    losses = np.where(np.isfinite(losses) & (losses < 1e29), losses, np.float32(0.0))
    return np.float32(losses.mean() / np.float32(L))


def kernel(input, label):
    from concourse.bass_utils import run_bass_kernel_spmd

    nc = get_program()
    in_maps = make_in_maps(input, label)
    res = run_bass_kernel_spmd(nc, in_maps, list(range(NCORES)))
    return finalize(res.results)


if __name__ == "__main__":
    rng = np.random.default_rng(0)
    x = rng.standard_normal((T, B, C), dtype=np.float32)
    lab = rng.integers(0, 5999, size=(B, L), dtype=np.int64)
    print(kernel(x, lab))
